# revision 2
# baseline (speedup 1.0000x reference)
"""Trainium2 Bass kernel v3 for nn_Attention_14190571946482.

Causal self-attention (diagonal masked too), with both projection folds:
  B[d',d]   = sum_u Wv[d',u] Wq[d,u]          (device, 16 mm)
  u_cT[d,k] = sum_d' B[d',d] xT[d',k]         (replaces kT; scores become
  scoreT[k,q] = u_cT . xq / sqrt(D)            x A x^T — qT projection gone)
  ctxdT[d,q] = sum_c x_c[k,d]^T attnT_c[k,q]  (context in the d-basis —
  out[q,u]  = ctxdT^T @ Wk / den               v projection gone)

Sharding: 8 cores = 4 batches x 2 roles; role r owns tiles {2j+r}.
Per core 2 groups of 4 slots: G2 = tiles {8..15} (chunks 0..15) first,
then G1 = tiles {0..7} (chunks 0..7). Role-dependent structure is input
data (qx gather, mask blocks, misc columns); the program is SPMD-identical.
Per chunk c only the live suffix of slots j >= j0 is scored; ctxdT
accumulates open-group (start only at c==0 full-width, skip_group_check)
into 4 [128,512] psum banks, one per d-tile, 4 slot columns each. Row 0
(fully masked) is blended to mean(v) on the final psum before normalize.
"""

import sys

sys.path.insert(0, "/opt/trn_rl_repo")

import numpy as np
import ml_dtypes

import concourse.bass as bass
import concourse.bacc as bacc
import concourse.mybir as mybir
from concourse.tile import TileContext
from concourse import bass_utils

BF16 = ml_dtypes.bfloat16

B, S, D, U = 4, 2048, 512, 512
P = 128
SCALE = 1.0 / float(np.sqrt(np.float32(D)))
GROUPS = [(8, 16), (0, 8)]  # (tbase, nchunks): G2 first, then G1
NSLOT = 8                   # output blocks: b=0..3 G2 slots, 4..7 G1
SPECIAL = 4                 # G1 slot 0 holds tiles (0,1): row-0 blend
NWARM = 8

_nc_cache = None


def build_nc():
    global _nc_cache
    if _nc_cache is not None:
        return _nc_cache

    f32 = mybir.dt.float32
    bf16 = mybir.dt.bfloat16

    nc = bacc.Bacc()
    xT_d = nc.declare_dram_parameter("xT", [D, S], bf16, isOutput=False)
    xo_d = nc.declare_dram_parameter("xo", [S, D], bf16, isOutput=False)
    qx_d = nc.declare_dram_parameter("qx", [D, NSLOT * P], bf16, isOutput=False)
    wqT_d = nc.declare_dram_parameter("wqT", [U, D], bf16, isOutput=False)
    wvT_d = nc.declare_dram_parameter("wvT", [U, D], bf16, isOutput=False)
    wk_d = nc.declare_dram_parameter("wk", [D, U], bf16, isOutput=False)
    # 16 frontier mask blocks [128,128]: G2 chunks 8..15, then G1 0..7.
    mm_d = nc.declare_dram_parameter("maskblk", [P, 16 * P], bf16, isOutput=False)
    # misc f32: [0,0] rsel0 (row-0 ctx factor), [0,1] rscale (1/S or 0),
    # cols 8..15: per-output-block sume column.
    ms_d = nc.declare_dram_parameter("misc", [P, 16], f32, isOutput=False)
    out_d = nc.declare_dram_parameter("out", [NSLOT * P, U], bf16, isOutput=True)

    with TileContext(nc) as tc:
        with (
            tc.tile_pool(name="cst", bufs=1) as cst,
            tc.tile_pool(name="work", bufs=4) as work,
            tc.tile_pool(name="small", bufs=8) as small,
            tc.tile_pool(name="psA", bufs=3, space="PSUM") as psA,
            tc.tile_pool(name="psC", bufs=4, space="PSUM") as psC,
            tc.tile_pool(name="psD", bufs=1, space="PSUM") as psD,
        ):
            # ---- on-chip constants ----
            wu = cst.tile([P, 512], bf16, tag="wu")
            nc.vector.memset(wu, 0.0)
            ones_c = cst.tile([P, 1], bf16, tag="ones")
            nc.gpsimd.memset(ones_c, 1.0)

            # ---- input DMAs. wvT+wqT lead (B depends on them), then the
            # xT g-slices feeding the u_cT loop; qx/wk/masks next; xo last
            # (ctxdT consumes it ~25us in) ----
            wvT_t = cst.tile([P, 4, D], bf16, tag="wvT")
            wqT_t = cst.tile([P, 4, D], bf16, tag="wqT")
            xT_t = cst.tile([P, 4, S], bf16, tag="xT")
            xT_r = xT_d.rearrange("(d p) s -> p d s", p=P)
            xo_t = cst.tile([P, 16, D], bf16, tag="xo")
            xo_r = xo_d.rearrange("(c p) d -> p c d", p=P)
            wk_t = cst.tile([P, 4, U], bf16, tag="wk")
            qx_t = cst.tile([P, 4, NSLOT * P], bf16, tag="qx")
            maskblk = cst.tile([P, 16 * P], bf16, tag="maskblk")
            misc = cst.tile([P, 16], f32, tag="misc")
            nc.sync.dma_start(out=wvT_t, in_=wvT_d.rearrange("(k p) d -> p k d", p=P))
            nc.scalar.dma_start(out=wqT_t, in_=wqT_d.rearrange("(k p) d -> p k d", p=P))
            nc.sync.dma_start(out=xT_t[:, :, 0:512], in_=xT_r[:, :, 0:512])
            nc.scalar.dma_start(out=xT_t[:, :, 512:1024], in_=xT_r[:, :, 512:1024])
            nc.sync.dma_start(out=xT_t[:, :, 1024:1536], in_=xT_r[:, :, 1024:1536])
            nc.scalar.dma_start(out=xT_t[:, :, 1536:2048], in_=xT_r[:, :, 1536:2048])
            nc.sync.dma_start(out=qx_t, in_=qx_d.rearrange("(d p) s -> p d s", p=P))
            nc.scalar.dma_start(out=wk_t, in_=wk_d.rearrange("(d p) u -> p d u", p=P))
            nc.sync.dma_start(out=maskblk, in_=mm_d[:, :])
            nc.sync.dma_start(out=misc, in_=ms_d[:, :])
            nc.scalar.dma_start(out=xo_t[:, 0:8, :], in_=xo_r[:, 0:8, :])
            nc.sync.dma_start(out=xo_t[:, 8:16, :], in_=xo_r[:, 8:16, :])

            # ---- PE warm-up: ramp the HAM clock while DMAs land ----
            for _ in range(NWARM):
                wups = psA.tile([P, 512], f32, tag="blk")
                nc.tensor.matmul(wups, lhsT=wu[:, :P], rhs=wu,
                                 start=True, stop=True)

            # ---- B = Wv @ Wq^T, tiles [d' part, d free] ----
            B_sb = cst.tile([P, 4, D], bf16, tag="Bsb")
            for t in range(4):
                ps = psA.tile([P, 512], f32, tag="blk")
                for ku in range(4):
                    nc.tensor.matmul(
                        ps,
                        lhsT=wvT_t[:, ku, t * P:(t + 1) * P],
                        rhs=wqT_t[:, ku, :],
                        start=(ku == 0), stop=(ku == 3),
                    )
                if t % 2 == 0:
                    nc.vector.tensor_copy(B_sb[:, t, :], ps)
                else:
                    nc.scalar.copy(B_sb[:, t, :], ps)

            # ---- u_cT [d, s]: per g-slice so compute follows the xT DMA ----
            uT = [cst.tile([P, S], bf16, tag=f"uT{m}", name=f"uT{m}")
                  for m in range(4)]
            ci = 0
            for g in range(4):
                for m in range(4):
                    ps = psA.tile([P, 512], f32, tag="blk")
                    for t in range(4):
                        nc.tensor.matmul(
                            ps,
                            lhsT=B_sb[:, t, m * P:(m + 1) * P],
                            rhs=xT_t[:, t, g * 512:(g + 1) * 512],
                            start=(t == 0), stop=(t == 3),
                        )
                    dst = uT[m][:, g * 512:(g + 1) * 512]
                    if ci % 2 == 0:
                        nc.vector.tensor_copy(dst, ps)
                    else:
                        nc.scalar.copy(dst, ps)
                    ci += 1

            # ---- mean-of-v (for the fully-masked global row 0) ----
            xs16 = []
            for d in range(4):
                xs = small.tile([P, 1], f32, tag="xs")
                nc.vector.reduce_sum(xs, xT_t[:, d, :], axis=mybir.AxisListType.X)
                x16 = small.tile([P, 1], bf16, tag="xs16")
                nc.vector.tensor_copy(x16, xs)
                xs16.append(x16)
            vm_ps = psA.tile([1, 512], f32, tag="blk")
            for d in range(4):
                nc.tensor.matmul(vm_ps, lhsT=xs16[d], rhs=wk_t[:, d, :],
                                 start=(d == 0), stop=(d == 3))
            vm_sb = cst.tile([1, 512], f32, tag="vm_sb")
            nc.vector.tensor_scalar_mul(vm_sb, vm_ps, misc[0:1, 1:2])

            # ---- phase 2: transposed-score attention, d-basis context ----
            # one psum bank holds all 8 slots' denominators (col b = 4g+j)
            den_t = psD.tile([P, 8], f32, tag="dent", name="dent")
            nc.vector.memset(den_t, 0.0)
            for g, (tbase, nchunks) in enumerate(GROUPS):
                cd_ps = [psC.tile([P, 512], f32, tag="ctx", name=f"cd{g}_{m}")
                         for m in range(4)]
                mask_base = 0 if g == 0 else 8
                for c in range(nchunks):
                    j0 = max(0, (c - tbase) // 2)
                    ncols = (4 - j0) * P
                    qoff = g * 512 + j0 * P
                    sc_ps = psA.tile([P, 512], f32, tag="blk")
                    for m in range(4):
                        nc.tensor.matmul(
                            sc_ps[:, :ncols],
                            lhsT=uT[m][:, c * P:(c + 1) * P],
                            rhs=qx_t[:, m, qoff:qoff + ncols],
                            start=(m == 0), stop=(m == 3),
                        )
                    attnT = work.tile([P, 512], bf16, tag="attnT")
                    nc.scalar.activation(
                        attnT[:, :ncols], sc_ps[:, :ncols],
                        mybir.ActivationFunctionType.Exp, scale=SCALE,
                    )
                    cl = c - tbase
                    if cl >= 0:
                        j = cl // 2
                        mb = (mask_base + cl) * P
                        sl = attnT[:, (j - j0) * P:(j - j0 + 1) * P]
                        nc.vector.tensor_mul(sl, sl, maskblk[:, mb:mb + P])
                    # den matmuls (1 col each; slot-last leads the rcp path)
                    fin = (cl % 2 == 1) and cl >= 0
                    jf = cl // 2 if fin else -1
                    for j in range(j0, 4):
                        b = 4 * g + j
                        blk = attnT[:, (j - j0) * P:(j - j0 + 1) * P]
                        nc.tensor.matmul(den_t[:, b:b + 1], lhsT=blk,
                                         rhs=ones_c, start=False,
                                         stop=(j == jf),
                                         skip_group_check=True)
                    # ctxdT accumulate: one matmul per d-tile over live cols
                    for m in range(4):
                        nc.tensor.matmul(
                            cd_ps[m][:, j0 * P:512],
                            lhsT=xo_t[:, c, m * P:(m + 1) * P],
                            rhs=attnT[:, :ncols],
                            start=(c == 0), stop=(c == nchunks - 1),
                            skip_group_check=True,
                        )
                    if fin:
                        j = jf
                        b = 4 * g + j
                        den = small.tile([P, 1], f32, tag="den")
                        nc.vector.tensor_add(den, den_t[:, b:b + 1],
                                             misc[:, 8 + b:9 + b])
                        rcp = small.tile([P, 1], f32, tag="rcp")
                        nc.vector.reciprocal(rcp, den)
                        # ctxdT slot j is final: copy to sbuf, GEMM with Wk
                        cds = work.tile([P, 4, P], bf16, tag="cds")
                        for m in range(4):
                            src = cd_ps[m][:, j * P:(j + 1) * P]
                            if m % 2 == 0:
                                nc.scalar.copy(cds[:, m, :], src)
                            else:
                                nc.vector.tensor_copy(cds[:, m, :], src)
                        out_ps = psA.tile([P, 512], f32, tag="blk")
                        for m in range(4):
                            nc.tensor.matmul(out_ps, lhsT=cds[:, m, :],
                                             rhs=wk_t[:, m, :],
                                             start=(m == 0), stop=(m == 3))
                        if b == SPECIAL:
                            # row 0 of role 0 = mean(v): on psum f32
                            nc.vector.tensor_scalar_mul(
                                out_ps[0:1, :], out_ps[0:1, :],
                                misc[0:1, 0:1])
                            nc.vector.tensor_add(
                                out_ps[0:1, :], out_ps[0:1, :], vm_sb)
                        ctx_sb = work.tile([P, 512], bf16, tag="ctxs")
                        for hh in range(2):
                            nc.scalar.activation(
                                ctx_sb[:, hh * 256:(hh + 1) * 256],
                                out_ps[:, hh * 256:(hh + 1) * 256],
                                mybir.ActivationFunctionType.Copy,
                                scale=rcp)
                            nc.sync.dma_start(
                                out=out_d[b * P:(b + 1) * P,
                                          hh * 256:(hh + 1) * 256],
                                in_=ctx_sb[:, hh * 256:(hh + 1) * 256])

    nc.compile()
    _nc_cache = nc
    return nc


def tile_of_block(b, r):
    """Global q-tile held by output block b on role r."""
    return (8 + 2 * b + r) if b < 4 else (2 * (b - 4) + r)


def host_inputs(query, Wq, Wv, Wk):
    """Build per-core input maps. query [B,S,D] f32; W* [D,U] f32."""
    wqT16 = np.ascontiguousarray(Wq.T).astype(BF16)
    wvT16 = np.ascontiguousarray(Wv.T).astype(BF16)
    wk16 = Wk.astype(BF16)

    p = np.arange(P)[:, None]   # kk within chunk
    f = np.arange(P)[None, :]   # q within tile
    tri = (p < f).astype(np.float32)        # diag block: kk < q valid
    ones_b = np.ones((P, P), np.float32)
    zeros_b = np.zeros((P, P), np.float32)

    masks = {}
    for r in range(2):
        blocks = []
        for g, (tbase, nchunks) in enumerate(GROUPS):
            for cl in range(8):
                # chunk c = tbase + cl, affected slot j = cl//2,
                # role tile t = tbase + 2*(cl//2) + r
                c = tbase + cl
                t = tbase + 2 * (cl // 2) + r
                if c < t:
                    blocks.append(ones_b)
                elif c == t:
                    blocks.append(tri)
                else:
                    blocks.append(zeros_b)
        masks[r] = np.concatenate(blocks, axis=1).astype(BF16)

    in_maps = []
    for core in range(8):
        b_, r = core // 2, core % 2
        xo = np.ascontiguousarray(query[b_]).astype(BF16)         # [S, D]
        xTb = np.ascontiguousarray(query[b_].T).astype(BF16)      # [D, S]
        cols = np.concatenate(
            [np.arange(P * tile_of_block(b, r), P * tile_of_block(b, r) + P)
             for b in range(NSLOT)]
        )
        qx = np.ascontiguousarray(xTb[:, cols])                   # [D, 1024]
        misc = np.zeros((P, 16), np.float32)
        misc[0, 0] = 0.0 if r == 0 else 1.0      # rsel0
        misc[0, 1] = (1.0 / S) if r == 0 else 0.0  # rscale
        if r == 0:
            misc[0, 8 + SPECIAL] = 1.0           # den fix for global row 0
        in_maps.append({
            "xT": xTb, "xo": xo, "qx": qx,
            "wqT": wqT16, "wvT": wvT16, "wk": wk16,
            "maskblk": masks[r], "misc": misc,
        })
    return in_maps


def assemble_output(results):
    """results: list of 8 dicts with 'out' [1024, 512] bf16."""
    out = np.zeros((B, S, U), np.float32)
    for core in range(8):
        b_, r = core // 2, core % 2
        o = np.asarray(results[core]["out"], dtype=np.float32)
        for b in range(NSLOT):
            t = tile_of_block(b, r)
            out[b_, P * t:P * (t + 1), :] = o[P * b:P * (b + 1), :]
    return out


def run(query, Wq, Wv, Wk, **kwargs):
    nc = build_nc()
    in_maps = host_inputs(
        np.asarray(query, np.float32), np.asarray(Wq, np.float32),
        np.asarray(Wv, np.float32), np.asarray(Wk, np.float32),
    )
    res = bass_utils.run_bass_kernel_spmd(nc, in_maps, list(range(8)), **kwargs)
    return assemble_output(res.results), res


def kernel(query, Wq, Wv, Wk):
    out, _ = run(query, Wq, Wv, Wk)
    return out


if __name__ == "__main__":
    rng = np.random.default_rng(0)
    q = rng.standard_normal((B, S, D), dtype=np.float32)
    scale = np.sqrt(2.0 / (D + U)).astype(np.float32)
    Wq = rng.standard_normal((D, U), dtype=np.float32) * scale
    Wv = rng.standard_normal((D, U), dtype=np.float32) * scale
    Wk = rng.standard_normal((D, U), dtype=np.float32) * scale
    out = kernel(q, Wq, Wv, Wk)
    print(out.shape, out.dtype, np.abs(out).mean())
